# revision 16
# baseline (speedup 1.0000x reference)
"""Trainium2 Bass kernel: distance-decay double-softmax attention.

Reference computation per (b, c) pair (L=256, D=512):
    qkv  = x @ w_qkv;  q,k,v = split(qkv)
    attn = softmax(q @ k.T * D_h^-0.5)
    h    = relu((attn + pos) @ w1 + b1);  w = h @ w2 + b2
    attn2= softmax(attn * exp(-dist / (2 w^2 + 1e-6)))
    out  = (attn2 @ v) @ w_out + b_out

Host-side algebraic folds (exact):
    dots = q k^T * s = x (s Wq Wk^T) x^T   -> M = s*Wq@Wk.T
    y    = attn2 @ (v w_out) + b_out       -> Wv' = Wv@w_out, b_out on host
    pos streamed raw and added on-device (GpSimd), so no P1 precompute.

Dtype strategy (rel-err budget 2e-2; measured ~3.4e-3 end-to-end):
bf16 for x/M/Wv'/t/E/attn2/v (PE runs bf16 at the same 1 cyc/row as
f32r but with half the LDWEIGHTS cost and half the DMA), fp8e4m3 +
DoubleRow (2x PE rate, K=256 single pass) for the width-MLP whose
effect on the final output is empirically insensitive (4e-5).

Scheduling: the per-superpair B-chain (transpose -> MLP -> negt -> wg
-> softmax2 -> transpose -> y) is a long cross-engine dependency chain.
Every PE step of it is interleaved with independent stage-A matmuls of
superpair sp+2, and the second softmax + B2 run one full period later
than B1, so no PE instruction ever waits on a fresh cross-engine hop.
Emission order per engine is chosen so the in-order DVE/Act queues
never head-of-line block a ready copy behind a waiting reduce.
"""

import sys
import numpy as np

sys.path.insert(0, "/opt/trn_rl_repo")

import concourse.bass as bass  # noqa: E402,F401
import concourse.mybir as mybir  # noqa: E402
from concourse import bacc  # noqa: E402
from concourse.tile import TileContext  # noqa: E402

F32 = mybir.dt.float32
BF16 = mybir.dt.bfloat16
F8 = mybir.dt.float8e4
AF = mybir.ActivationFunctionType
ALU = mybir.AluOpType
DR = mybir.MatmulPerfMode.DoubleRow

B, C, L, D = 8, 16, 256, 512
NCORES = 8
CH_PER_CORE = C // NCORES          # 2
NSP = (B // 2) * CH_PER_CORE       # 8 superpairs per core
P = 128
FP = 2 * L                         # 512: two pairs packed along free dim
FP2 = 2 * FP                       # 1024: both i-tiles packed
DT = D // P                        # 4
LT = L // P                        # 2
SCALE = float(64 ** -0.5)          # DIM_HEAD ** -0.5


class _Ctx:
    pass


# ---------------- stage-A pieces (superpair sp) ----------------

def _tT_group(g, sp, ets):
    nc, pp = g.nc, g.pp
    MM = nc.tensor.matmul
    st = g.state[sp]
    for et in ets:
        ps = pp.tile([P, FP], F32, tag="ps", name=f"ps_t{sp}_{et}")
        for dt in range(DT):
            MM(ps[:, :], g.m_sb[dt][:, et * P:(et + 1) * P], st.xt[dt][:, :],
               start=(dt == 0), stop=(dt == DT - 1))
        t = g.apool.tile([P, FP], BF16, tag=f"tT{et}", name=f"tT{sp}_{et}")
        nc.vector.tensor_copy(t[:, :], ps[:, :])
        st.tT.append(t)


def a_xt_half1(g, sp):
    """xt DMA; first half of t^T = (x M)^T."""
    nc = g.nc
    st = g.state[sp] = _Ctx()
    xt = []
    for dt in range(DT):
        t = g.apool.tile([P, FP], BF16, tag=f"xt{dt}", name=f"xt{sp}_{dt}")
        nc.sync.dma_start(out=t[:, :],
                          in_=g.h["x_t"][sp, dt * P:(dt + 1) * P, :])
        xt.append(t)
    st.xt = xt
    st.tT = []
    _tT_group(g, sp, range(0, DT // 2))


def a_xt_half2(g, sp):
    _tT_group(g, sp, range(DT // 2, DT))


def a_v(g, sp):
    """v = x Wv' (16 MMs) + copies (b_out added on host)."""
    nc, pp = g.nc, g.pp
    MM = nc.tensor.matmul
    st = g.state[sp]
    xt = st.xt
    v_sb = [[None] * LT for _ in range(2)]
    for pi in range(2):
        for lt in range(LT):
            ps = pp.tile([P, D], F32, tag="ps", name=f"ps_v{sp}_{pi}{lt}")
            for dt in range(DT):
                MM(ps[:, :],
                   xt[dt][:, pi * L + lt * P: pi * L + (lt + 1) * P],
                   g.wv_sb[dt][:, :],
                   start=(dt == 0), stop=(dt == DT - 1))
            t = g.vpool.tile([P, D], BF16, tag=f"v{pi}{lt}",
                             name=f"v{sp}_{pi}{lt}")
            if pi == 0:
                nc.scalar.activation(t[:, :], ps[:, :], AF.Copy)
            else:
                nc.vector.tensor_copy(t[:, :], ps[:, :])
            v_sb[pi][lt] = t
    st.v = v_sb


def a_dots(g, sp):
    """dots (16 MMs), E = exp(dots) packed [P,(it,pi,m)], s14, r14."""
    nc, pp = g.nc, g.pp
    MM = nc.tensor.matmul
    st = g.state[sp]
    xt, tT = st.xt, st.tT
    E = g.epool.tile([P, FP2], BF16, tag="E", name=f"E_{sp}")
    for it in range(LT):
        ps = pp.tile([P, FP], F32, tag="ps", name=f"ps_d{sp}_{it}")
        for pi in range(2):
            o = ps[:, pi * L:(pi + 1) * L]
            for et in range(DT):
                MM(o,
                   tT[et][:, pi * L + it * P: pi * L + (it + 1) * P],
                   xt[et][:, pi * L:(pi + 1) * L],
                   start=(et == 0), stop=(et == DT - 1))
        nc.scalar.activation(E[:, it * FP:(it + 1) * FP], ps[:, :], AF.Exp)
    st.E = E


def a_sm1(g, sp):
    """s14 = rowsums(E) (one wide DVE reduce); r14 = 1/s14."""
    nc = g.nc
    st = g.state[sp]
    s14 = g.apool.tile([P, 4], F32, tag="s14", name=f"s14_{sp}")
    nc.vector.tensor_reduce(
        s14[:, :], st.E[:, :].rearrange("q (c m) -> q c m", c=4),
        axis=mybir.AxisListType.X, op=ALU.add)
    r14 = g.apool.tile([P, 4], F32, tag="r14", name=f"r14_{sp}")
    nc.vector.reciprocal(r14[:, :], s14[:, :])
    st.r14 = r14


# ---------------- stage-B pieces ----------------

def _bcast4(r):
    """[P,4] per-(it,pi) scalars -> broadcast AP matching [P,(it,pi,m)]."""
    return r[:, :].unsqueeze(2).broadcast_to((P, 4, L))


def b1_pool(g, sp):
    """attn = E*r1; apw = attn + pos (GpSimd wide ops)."""
    nc = g.nc
    st = g.state[sp]
    ci = sp // (NSP // CH_PER_CORE)
    attn = g.p3.tile([P, FP2], BF16, tag="attn", name=f"attn_{sp}")
    nc.gpsimd.tensor_mul(attn[:, :], st.E[:, :], _bcast4(st.r14))
    st.attn = attn
    apw = g.p3.tile([P, FP2], BF16, tag="apw", name=f"apw_{sp}")
    pos_b = (g.pos_sb[ci][:, :].rearrange("q (a m) -> q a m", a=2)
             .unsqueeze(2).broadcast_to((P, 2, 2, L)))
    nc.gpsimd.tensor_add(apw[:, :], attn[:, :], pos_b)
    st.apw = apw


def b1_t1(g, sp):
    """apw^T -> PSUM bf16 [P,(kk,i)=1024]; copy to fp8 aT8."""
    nc, pp = g.nc, g.pp
    st = g.state[sp]
    aps = pp.tile([P, FP2], BF16, tag="ps", name=f"ps_tA{sp}")
    for mt in range(LT):
        for pi in range(2):
            for it in range(LT):
                nc.tensor.transpose(
                    aps[:, mt * FP + pi * L + it * P:
                        mt * FP + pi * L + (it + 1) * P],
                    st.apw[:, it * FP + pi * L + mt * P:
                           it * FP + pi * L + (mt + 1) * P],
                    g.id_sb[:, :])
    aT8 = g.p2p.tile([P, 2, FP], F8, tag="aT8", name=f"aT8_{sp}")
    nc.vector.tensor_copy(aT8[:, :, :], aps[:, :])
    st.aT8 = aT8


def b1_h(g, sp):
    """h^T = relu(w1^T apw^T + b1): 2 fp8 DoubleRow MMs + Act relu."""
    nc, pp = g.nc, g.pp
    MM = nc.tensor.matmul
    st = g.state[sp]
    hT8 = g.p2p.tile([P, 2, FP], F8, tag="hT8", name=f"hT8_{sp}")
    for jt in range(LT):
        ps = pp.tile([P, FP], F32, tag="ps", name=f"ps_h{sp}_{jt}")
        MM(ps[:, :], g.w1_sb[:, :, jt * P:(jt + 1) * P], st.aT8[:, :, :],
           start=True, stop=True, perf_mode=DR)
        nc.scalar.activation(hT8[:, jt, :], ps[:, :], AF.Relu,
                             bias=g.b1_sb[:, jt:jt + 1])
    st.hT8 = hT8


def b1_w(g, sp):
    """w^T = w2^T h^T (one DR MM, rows 0..1 valid) + DVE copy."""
    nc, pp = g.nc, g.pp
    st = g.state[sp]
    wps = pp.tile([P, FP], F32, tag="ps", name=f"ps_w{sp}")
    nc.tensor.matmul(wps[:, :], g.w2_sb[:, :, :], st.hT8[:, :, :],
                     start=True, stop=True, perf_mode=DR)
    wTs = g.p2p.tile([2, FP], BF16, tag="wTs", name=f"wTs_{sp}")
    nc.vector.tensor_copy(wTs[:, :], wps[0:2, :])
    st.wTs = wTs


def b1_wtp(g, sp):
    """w back to [i,8]; negt; wg = exp(dist*negt); p2 = attn*wg (Pool)."""
    nc, pp = g.nc, g.pp
    st = g.state[sp]
    wtp = pp.tile([P, 8], BF16, tag="ps", name=f"ps_wt{sp}")
    for pi in range(2):
        for it in range(LT):
            c = it * 2 + pi
            nc.tensor.transpose(
                wtp[:, 2 * c:2 * c + 2],
                st.wTs[0:2, pi * L + it * P: pi * L + (it + 1) * P],
                g.id_sb[0:2, 0:2])
    w4 = g.p2p.tile([P, 8], F32, tag="w4", name=f"w4_{sp}")
    nc.scalar.activation(w4[:, :], wtp[:, :], AF.Square, bias=g.b2_sb[:, 0:1])
    nc.vector.tensor_scalar(w4[:, :], w4[:, :], -2.0, -1e-6, ALU.mult, ALU.add)
    negt = g.p2p.tile([P, 8], F32, tag="negt", name=f"negt_{sp}")
    nc.vector.reciprocal(negt[:, :], w4[:, :])
    wg = g.p3.tile([P, FP2], BF16, tag="wg", name=f"wg_{sp}")
    for it in range(LT):
        for pi in range(2):
            c = it * 2 + pi
            sl = slice(it * FP + pi * L, it * FP + (pi + 1) * L)
            nc.scalar.activation(wg[:, sl], g.dist_sb[it][:, :], AF.Exp,
                                 scale=negt[:, 2 * c:2 * c + 1])
    nc.gpsimd.tensor_mul(wg[:, :], st.attn[:, :], wg[:, :])
    st.wg = wg


def b1_sm2(g, sp):
    """E2 = exp(p2); s24 (DVE reduce); attn2 = E2*r24 (Pool)."""
    nc = g.nc
    st = g.state[sp]
    wg = st.wg
    for it in range(LT):
        sl = slice(it * FP, (it + 1) * FP)
        nc.scalar.activation(wg[:, sl], wg[:, sl], AF.Exp)
    s24 = g.p2p.tile([P, 4], F32, tag="s24", name=f"s24_{sp}")
    nc.vector.tensor_reduce(
        s24[:, :], wg[:, :].rearrange("q (c m) -> q c m", c=4),
        axis=mybir.AxisListType.X, op=ALU.add)
    nc.sync.dma_start(out=g.h["s24o"][sp, :, :], in_=s24[:, :])


def b2_t(g, sp):
    """attn2^T -> PSUM bf16; copy to SBUF."""
    nc, pp = g.nc, g.pp
    st = g.state[sp]
    a2ps = pp.tile([P, FP2], BF16, tag="ps", name=f"ps_tB{sp}")
    for mt in range(LT):
        for pi in range(2):
            for it in range(LT):
                nc.tensor.transpose(
                    a2ps[:, mt * FP + pi * L + it * P:
                         mt * FP + pi * L + (it + 1) * P],
                    st.wg[:, it * FP + pi * L + mt * P:
                          it * FP + pi * L + (mt + 1) * P],
                    g.id_sb[:, :])
    a2T = g.p2p.tile([P, FP2], BF16, tag="a2T", name=f"a2T_{sp}")
    nc.vector.tensor_copy(a2T[:, :], a2ps[:, :])
    st.a2T = a2T


def b2_y(g, sp):
    """y = attn2 @ v (8 MMs), copy out bf16, DMA."""
    nc, pp = g.nc, g.pp
    MM = nc.tensor.matmul
    st = g.state[sp]
    for pi in range(2):
        for it in range(LT):
            ps = g.ypp.tile([P, D], F32, tag="psy", name=f"ps_y{sp}_{pi}{it}")
            for mt in range(LT):
                MM(ps[:, :],
                   st.a2T[:, mt * FP + pi * L + it * P:
                          mt * FP + pi * L + (it + 1) * P],
                   st.v[pi][mt][:, :],
                   start=(mt == 0), stop=(mt == LT - 1))
            yt = g.ypool.tile([P, D], BF16, tag=f"y{pi}{it}",
                              name=f"y{sp}_{pi}{it}")
            if (pi + it) % 2 == 0:
                nc.vector.tensor_copy(yt[:, :], ps[:, :])
            else:
                nc.scalar.activation(yt[:, :], ps[:, :], AF.Copy)
            eng = nc.sync if (pi + it) % 2 == 0 else nc.scalar
            eng.dma_start(
                out=g.h["out"][sp, pi * L + it * P: pi * L + (it + 1) * P, :],
                in_=yt[:, :])


def _emit(nc, tc, h):
    import contextlib
    g = _Ctx()
    g.nc, g.h = nc, h
    g.state = {}

    with contextlib.ExitStack() as ex:
        cpool = ex.enter_context(tc.tile_pool(name="consts", bufs=1))
        g.apool = ex.enter_context(tc.tile_pool(name="astream", bufs=3))
        g.vpool = ex.enter_context(tc.tile_pool(name="vstream", bufs=4))
        g.epool = ex.enter_context(tc.tile_pool(name="estream", bufs=4))
        g.p3 = ex.enter_context(tc.tile_pool(name="bstream3", bufs=3))
        g.p2p = ex.enter_context(tc.tile_pool(name="bstream2", bufs=2))
        g.ypool = ex.enter_context(tc.tile_pool(name="yout", bufs=2))
        g.pp = ex.enter_context(tc.tile_pool(name="ps", bufs=6, space="PSUM"))
        g.ypp = ex.enter_context(tc.tile_pool(name="psy", bufs=2, space="PSUM"))

        def cload(name, shape, dt_, src):
            t = cpool.tile(shape, dt_, tag=name, name=name)
            t_all = t[:, :, :] if len(shape) == 3 else t[:, :]
            nc.sync.dma_start(out=t_all, in_=src)
            return t

        def fill(n):
            fp = g.pp.tile([P, FP], F32, tag="ps", name=f"fill{g.nfill}")
            g.nfill += 1
            for _ in range(n):
                nc.tensor.matmul(fp[:, :], g.id_sb[:, :], g.m_sb[0][:, :],
                                 start=True, stop=True)
        g.nfill = 0

        # Identity first, then warmup matmuls during the input-DMA head
        # (lifts the cold HAM pstate while the PE would idle anyway).
        warm_in = cpool.tile([P, P], BF16, tag="warm_in", name="warm_in")
        nc.gpsimd.memset(warm_in[:, :], 1.0)
        g.id_sb = cload("identb", [P, P], BF16, h["identb"][:, :])
        g.id8_sb = cload("ident8", [P, P], F8, h["ident8"][:, :])
        warm_ps = g.pp.tile([P, P], F32, tag="ps", name="warmup_ps")
        for wi in range(28):
            nc.tensor.matmul(warm_ps[:, :], warm_in[:, :], warm_in[:, :],
                             start=True, stop=True)
        g.m_sb = [cload(f"m{dt}", [P, D], BF16, h["m"][dt * P:(dt + 1) * P, :])
                  for dt in range(DT)]

        # ---- pipeline fill ----
        a_xt_half1(g, 0)
        g.wv_sb = [cload(f"wv{dt}", [P, D], BF16,
                         h["wv"][dt * P:(dt + 1) * P, :])
                   for dt in range(DT)]
        g.w1_sb = cload("w1dr", [P, 2, L], F8, h["w1_dr"][:, :, :])
        g.w2_sb = cload("w2dr", [P, 2, P], F8, h["w2_dr"][:, :, :])
        g.b1_sb = cload("b1r", [P, 2], F32, h["b1r"][:, :])
        g.b2_sb = cload("b2r", [P, 1], F32, h["b2r"][:, :])
        g.pos_sb = [cload(f"pos{ci}", [P, FP], BF16, h["posb"][ci, :, :])
                    for ci in range(CH_PER_CORE)]
        g.dist_sb = [cload(f"dist{it}", [P, L], F32, h["distb"][it, :, :])
                     for it in range(LT)]
        a_xt_half2(g, 0)
        a_v(g, 0)
        a_dots(g, 0)
        a_sm1(g, 0)
        a_xt_half1(g, 1)
        a_xt_half2(g, 1)
        a_v(g, 1)
        a_dots(g, 1)
        b1_pool(g, 0)

        # ---- steady-state periods ----
        # Period k PE stream: B1(k) chain steps interleaved with A(k+2)
        # matmul blocks; B2 runs for superpair k-1 (one full period of
        # slack for the softmax2 cross-engine chain).  Engine queues are
        # ordered so ready ops never wait behind blocked ones.
        for k in range(NSP - 2):
            b1_t1(g, k)
            if k + 2 < NSP:
                a_xt_half1(g, k + 2)
            else:
                fill(6)
            if k + 1 < NSP:
                a_sm1(g, k + 1)
            if k >= 1:
                b1_sm2(g, k - 1)
            b1_h(g, k)
            if k + 2 < NSP:
                a_xt_half2(g, k + 2)
            else:
                fill(4)
            if k + 1 < NSP:
                b1_pool(g, k + 1)
            if k + 2 < NSP:
                a_v(g, k + 2)
            else:
                fill(10)
            b1_w(g, k)
            if k >= 1:
                b2_t(g, k - 1)
            b1_wtp(g, k)
            if k + 2 < NSP:
                a_dots(g, k + 2)
            else:
                fill(6)
            if k >= 1:
                b2_y(g, k - 1)

        # ---- period 6: B1(6) and B1(7) interleaved (hand-scheduled) ----
        q = NSP - 1
        b1_t1(g, q - 1)
        fill(2)
        a_sm1(g, q)
        b1_sm2(g, q - 2)
        b1_h(g, q - 1)
        fill(2)
        b1_pool(g, q)
        b1_w(g, q - 1)
        b2_t(g, q - 2)
        b1_wtp(g, q - 1)
        b2_y(g, q - 2)
        b1_t1(g, q)
        fill(4)
        b1_h(g, q)
        fill(2)
        b1_w(g, q)
        fill(3)
        b1_wtp(g, q)
        # ---- period 7: drain both B2 chains ----
        b1_sm2(g, q - 1)
        fill(6)
        b2_t(g, q - 1)
        b1_sm2(g, q)
        b2_y(g, q - 1)
        b2_t(g, q)
        b2_y(g, q)


def build_nc():
    nc = bacc.Bacc("TRN2", target_bir_lowering=False, debug=False,
                   enable_asserts=False)
    h = {}
    h["x_t"] = nc.declare_dram_parameter("x_t", [NSP, D, FP], BF16, False)
    h["m"] = nc.declare_dram_parameter("m", [D, D], BF16, False)
    h["wv"] = nc.declare_dram_parameter("wv", [D, D], BF16, False)
    h["w1_dr"] = nc.declare_dram_parameter("w1_dr", [P, 2, L], F8, False)
    h["w2_dr"] = nc.declare_dram_parameter("w2_dr", [P, 2, P], F8, False)
    h["b1r"] = nc.declare_dram_parameter("b1r", [P, 2], F32, False)
    h["b2r"] = nc.declare_dram_parameter("b2r", [P, 1], F32, False)
    h["posb"] = nc.declare_dram_parameter(
        "posb", [CH_PER_CORE, P, FP], BF16, False)
    h["distb"] = nc.declare_dram_parameter("distb", [LT, P, L], F32, False)
    h["identb"] = nc.declare_dram_parameter("identb", [P, P], BF16, False)
    h["ident8"] = nc.declare_dram_parameter("ident8", [P, P], F8, False)
    h["out"] = nc.declare_dram_parameter("out", [NSP, FP, D], BF16, True)
    h["s24o"] = nc.declare_dram_parameter("s24o", [NSP, P, 4], F32, True)

    with TileContext(nc) as tc:
        _emit(nc, tc, h)
    nc.compile()
    return nc


def make_in_maps(x, w_qkv, pos_emb, w1, b1, w2, b2, w_out, b_out):
    import ml_dtypes
    BFNP = ml_dtypes.bfloat16
    F8NP = mybir.dt.np(F8)
    f = lambda a: np.asarray(a, dtype=np.float32)
    x, w_qkv, pos_emb = f(x), f(w_qkv), f(pos_emb)
    w1, b1, w2, b2, w_out, b_out = f(w1), f(b1), f(w2), f(b2), f(w_out), f(b_out)

    wq, wk, wv = w_qkv[:, :D], w_qkv[:, D:2 * D], w_qkv[:, 2 * D:]
    m = (SCALE * (wq.astype(np.float64) @ wk.astype(np.float64).T)
         ).astype(BFNP)
    wvp = (wv.astype(np.float64) @ w_out.astype(np.float64)).astype(BFNP)

    # DoubleRow-packed MLP weights: [p, kk, j] = w[kk*128+p, j]
    w1_dr = np.ascontiguousarray(
        w1.reshape(2, P, L).transpose(1, 0, 2)).astype(F8NP)
    w2_dr = np.zeros((P, 2, P), np.float32)
    w2_dr[:, :, 0:2] = np.broadcast_to(
        w2.reshape(2, P, 1).transpose(1, 0, 2), (P, 2, 2))
    w2_dr = w2_dr.astype(F8NP)
    b1r = np.ascontiguousarray(b1.reshape(2, P).T)
    b2r = np.full((P, 1), b2.reshape(-1)[0], np.float32)

    # pos per channel: [128, (it, m)=512]
    posb_all = pos_emb[0].reshape(C, LT, P, L).transpose(0, 2, 1, 3)
    posb_all = np.ascontiguousarray(posb_all).reshape(C, P, FP).astype(BFNP)

    idx = np.arange(L, dtype=np.float32)
    dist = (idx[None, :] - idx[:, None]) ** 2
    distb = np.ascontiguousarray(dist.reshape(LT, P, L))

    common = {
        "m": np.ascontiguousarray(m),
        "wv": np.ascontiguousarray(wvp),
        "w1_dr": w1_dr,
        "w2_dr": w2_dr,
        "b1r": b1r,
        "b2r": b2r,
        "distb": distb,
        "identb": np.eye(P, dtype=BFNP),
        "ident8": np.eye(P, dtype=F8NP),
    }
    xb = x.astype(BFNP)
    in_maps = []
    for core in range(NCORES):
        x_t = np.empty((NSP, D, FP), BFNP)
        posb = np.empty((CH_PER_CORE, P, FP), BFNP)
        for ci in range(CH_PER_CORE):
            ch = core * CH_PER_CORE + ci
            posb[ci] = posb_all[ch]
            for bp in range(B // 2):
                s = ci * (B // 2) + bp
                x_t[s, :, :L] = xb[2 * bp, ch].T
                x_t[s, :, L:] = xb[2 * bp + 1, ch].T
        mcore = dict(common)
        mcore["x_t"] = x_t
        mcore["posb"] = posb
        in_maps.append(mcore)
    return in_maps


def assemble_out(results, b_out=None):
    """results: per-core dicts with 'out' [NSP, FP, D] (unnormalized
    E2 @ v) and 's24o' [NSP, P, 4] softmax2 row sums; normalize here."""
    y = np.empty((B, C, L, D), np.float32)
    for core in range(NCORES):
        o = np.asarray(results[core]["out"], np.float32)
        s24 = np.asarray(results[core]["s24o"], np.float32)
        # rows i = pi*L + it*P + p  <->  s24[:, p, it*2+pi]
        rs = s24.reshape(NSP, 1, P, 2, 2).transpose(0, 4, 3, 2, 1)
        o = o.reshape(NSP, 2, LT, P, D) / rs
        o = o.reshape(NSP, FP, D)
        for ci in range(CH_PER_CORE):
            ch = core * CH_PER_CORE + ci
            for bp in range(B // 2):
                s = ci * (B // 2) + bp
                y[2 * bp, ch] = o[s, :L, :]
                y[2 * bp + 1, ch] = o[s, L:, :]
    if b_out is not None:
        y += np.asarray(b_out, np.float32).reshape(1, 1, 1, D)
    return y


_NC = None
LAST_RESULT = None


def kernel(x, w_qkv, pos_emb, w1, b1, w2, b2, w_out, b_out):
    global _NC, LAST_RESULT
    from concourse.bass_utils import run_bass_kernel_spmd

    if _NC is None:
        _NC = build_nc()
    in_maps = make_in_maps(x, w_qkv, pos_emb, w1, b1, w2, b2, w_out, b_out)
    res = run_bass_kernel_spmd(_NC, in_maps, core_ids=list(range(NCORES)))
    LAST_RESULT = res
    return assemble_out(res.results, b_out=b_out)


# revision 17
# speedup vs baseline: 1.1963x; 1.1963x over previous
"""Trainium2 Bass kernel: distance-decay double-softmax attention.

Reference computation per (b, c) pair (L=256, D=512):
    qkv  = x @ w_qkv;  q,k,v = split(qkv)
    attn = softmax(q @ k.T * D_h^-0.5)
    h    = relu((attn + pos) @ w1 + b1);  w = h @ w2 + b2
    attn2= softmax(attn * exp(-dist / (2 w^2 + 1e-6)))
    out  = (attn2 @ v) @ w_out + b_out

Host-side algebraic folds (exact):
    dots = q k^T * s = x (s Wq Wk^T) x^T   -> M = s*Wq@Wk.T
    y    = attn2 @ (v w_out) + b_out       -> Wv' = Wv@w_out, b_out on host
    pos streamed raw and added on-device (GpSimd), so no P1 precompute.

Dtype strategy (rel-err budget 2e-2; measured ~3.4e-3 end-to-end):
bf16 for x/M/Wv'/t/E/attn2/v (PE runs bf16 at the same 1 cyc/row as
f32r but with half the LDWEIGHTS cost and half the DMA), fp8e4m3 +
DoubleRow (2x PE rate, K=256 single pass) for the width-MLP whose
effect on the final output is empirically insensitive (4e-5).

Scheduling: the per-superpair B-chain (transpose -> MLP -> negt -> wg
-> softmax2 -> transpose -> y) is a long cross-engine dependency chain.
Every PE step of it is interleaved with independent stage-A matmuls of
superpair sp+2, and the second softmax + B2 run one full period later
than B1, so no PE instruction ever waits on a fresh cross-engine hop.
Emission order per engine is chosen so the in-order DVE/Act queues
never head-of-line block a ready copy behind a waiting reduce.
"""

import sys
import numpy as np

sys.path.insert(0, "/opt/trn_rl_repo")

import concourse.bass as bass  # noqa: E402,F401
import concourse.mybir as mybir  # noqa: E402
from concourse import bacc  # noqa: E402
from concourse.tile import TileContext  # noqa: E402

F32 = mybir.dt.float32
BF16 = mybir.dt.bfloat16
F8 = mybir.dt.float8e4
AF = mybir.ActivationFunctionType
ALU = mybir.AluOpType
DR = mybir.MatmulPerfMode.DoubleRow

B, C, L, D = 8, 16, 256, 512
NCORES = 8
CH_PER_CORE = C // NCORES          # 2
NSP = (B // 2) * CH_PER_CORE       # 8 superpairs per core
P = 128
FP = 2 * L                         # 512: two pairs packed along free dim
FP2 = 2 * FP                       # 1024: both i-tiles packed
DT = D // P                        # 4
LT = L // P                        # 2
SCALE = float(64 ** -0.5)          # DIM_HEAD ** -0.5


class _Ctx:
    pass


# ---------------- stage-A pieces (superpair sp) ----------------

def _tT_group(g, sp, ets):
    nc, pp = g.nc, g.pp
    MM = nc.tensor.matmul
    st = g.state[sp]
    for et in ets:
        ps = pp.tile([P, FP], F32, tag="ps", name=f"ps_t{sp}_{et}")
        for dt in range(DT):
            MM(ps[:, :], g.m_sb[dt][:, et * P:(et + 1) * P], st.xt[dt][:, :],
               start=(dt == 0), stop=(dt == DT - 1))
        t = g.apool.tile([P, FP], BF16, tag=f"tT{et}", name=f"tT{sp}_{et}")
        nc.vector.tensor_copy(t[:, :], ps[:, :])
        st.tT.append(t)


def a_xt_half1(g, sp):
    """xt DMA; first half of t^T = (x M)^T."""
    nc = g.nc
    st = g.state[sp] = _Ctx()
    xt = []
    for dt in range(DT):
        t = g.apool.tile([P, FP], BF16, tag=f"xt{dt}", name=f"xt{sp}_{dt}")
        nc.sync.dma_start(out=t[:, :],
                          in_=g.h["x_t"][sp, dt * P:(dt + 1) * P, :])
        xt.append(t)
    st.xt = xt
    st.tT = []
    _tT_group(g, sp, range(0, DT // 2))


def a_xt_half2(g, sp):
    _tT_group(g, sp, range(DT // 2, DT))


def a_v(g, sp):
    """v = x Wv' (16 MMs) + copies (b_out added on host)."""
    nc, pp = g.nc, g.pp
    MM = nc.tensor.matmul
    st = g.state[sp]
    xt = st.xt
    v_sb = [[None] * LT for _ in range(2)]
    for pi in range(2):
        for lt in range(LT):
            ps = pp.tile([P, D], F32, tag="ps", name=f"ps_v{sp}_{pi}{lt}")
            for dt in range(DT):
                MM(ps[:, :],
                   xt[dt][:, pi * L + lt * P: pi * L + (lt + 1) * P],
                   g.wv_sb[dt][:, :],
                   start=(dt == 0), stop=(dt == DT - 1))
            t = g.vpool.tile([P, D], BF16, tag=f"v{pi}{lt}",
                             name=f"v{sp}_{pi}{lt}")
            if pi == 0:
                nc.scalar.activation(t[:, :], ps[:, :], AF.Copy)
            else:
                nc.vector.tensor_copy(t[:, :], ps[:, :])
            v_sb[pi][lt] = t
    st.v = v_sb


def a_dots(g, sp):
    """dots (16 MMs), E = exp(dots) packed [P,(it,pi,m)], s14, r14."""
    nc, pp = g.nc, g.pp
    MM = nc.tensor.matmul
    st = g.state[sp]
    xt, tT = st.xt, st.tT
    E = g.epool.tile([P, FP2], BF16, tag="E", name=f"E_{sp}")
    for it in range(LT):
        ps = pp.tile([P, FP], F32, tag="ps", name=f"ps_d{sp}_{it}")
        for pi in range(2):
            o = ps[:, pi * L:(pi + 1) * L]
            for et in range(DT):
                MM(o,
                   tT[et][:, pi * L + it * P: pi * L + (it + 1) * P],
                   xt[et][:, pi * L:(pi + 1) * L],
                   start=(et == 0), stop=(et == DT - 1))
        nc.scalar.activation(E[:, it * FP:(it + 1) * FP], ps[:, :], AF.Exp)
    st.E = E


def a_sm1(g, sp):
    """s14 = rowsums(E) (one wide DVE reduce); r14 = 1/s14."""
    nc = g.nc
    st = g.state[sp]
    s14 = g.apool.tile([P, 4], F32, tag="s14", name=f"s14_{sp}")
    nc.vector.tensor_reduce(
        s14[:, :], st.E[:, :].rearrange("q (c m) -> q c m", c=4),
        axis=mybir.AxisListType.X, op=ALU.add)
    r14 = g.apool.tile([P, 4], F32, tag="r14", name=f"r14_{sp}")
    nc.vector.reciprocal(r14[:, :], s14[:, :])
    st.r14 = r14


# ---------------- stage-B pieces ----------------

def _bcast4(r):
    """[P,4] per-(it,pi) scalars -> broadcast AP matching [P,(it,pi,m)]."""
    return r[:, :].unsqueeze(2).broadcast_to((P, 4, L))


def b1_pool(g, sp):
    """attn = E*r1; apw = attn + pos (GpSimd wide ops)."""
    nc = g.nc
    st = g.state[sp]
    ci = sp // (NSP // CH_PER_CORE)
    attn = g.p3.tile([P, FP2], BF16, tag="attn", name=f"attn_{sp}")
    nc.gpsimd.tensor_mul(attn[:, :], st.E[:, :], _bcast4(st.r14))
    st.attn = attn
    apw = g.p3.tile([P, FP2], BF16, tag="apw", name=f"apw_{sp}")
    pos_b = (g.pos_sb[ci][:, :].rearrange("q (a m) -> q a m", a=2)
             .unsqueeze(2).broadcast_to((P, 2, 2, L)))
    nc.gpsimd.tensor_add(apw[:, :], attn[:, :], pos_b)
    st.apw = apw


def b1_t1(g, sp):
    """apw^T -> PSUM bf16 [P,(kk,i)=1024]; copy to fp8 aT8."""
    nc, pp = g.nc, g.pp
    st = g.state[sp]
    aps = pp.tile([P, FP2], BF16, tag="ps", name=f"ps_tA{sp}")
    for mt in range(LT):
        for pi in range(2):
            for it in range(LT):
                nc.tensor.transpose(
                    aps[:, mt * FP + pi * L + it * P:
                        mt * FP + pi * L + (it + 1) * P],
                    st.apw[:, it * FP + pi * L + mt * P:
                           it * FP + pi * L + (mt + 1) * P],
                    g.id_sb[:, :])
    aT8 = g.p2p.tile([P, 2, FP], F8, tag="aT8", name=f"aT8_{sp}")
    nc.vector.tensor_copy(aT8[:, :, :], aps[:, :])
    st.aT8 = aT8


def b1_h(g, sp):
    """h^T = relu(w1^T apw^T + b1): 2 fp8 DoubleRow MMs + Act relu."""
    nc, pp = g.nc, g.pp
    MM = nc.tensor.matmul
    st = g.state[sp]
    hT8 = g.p2p.tile([P, 2, FP], F8, tag="hT8", name=f"hT8_{sp}")
    for jt in range(LT):
        ps = pp.tile([P, FP], F32, tag="ps", name=f"ps_h{sp}_{jt}")
        MM(ps[:, :], g.w1_sb[:, :, jt * P:(jt + 1) * P], st.aT8[:, :, :],
           start=True, stop=True, perf_mode=DR)
        nc.scalar.activation(hT8[:, jt, :], ps[:, :], AF.Relu,
                             bias=g.b1_sb[:, jt:jt + 1])
    st.hT8 = hT8


def b1_w(g, sp):
    """w^T = w2^T h^T (one DR MM, rows 0..1 valid) + DVE copy."""
    nc, pp = g.nc, g.pp
    st = g.state[sp]
    wps = pp.tile([P, FP], F32, tag="ps", name=f"ps_w{sp}")
    nc.tensor.matmul(wps[:, :], g.w2_sb[:, :, :], st.hT8[:, :, :],
                     start=True, stop=True, perf_mode=DR)
    wTs = g.p2p.tile([2, FP], BF16, tag="wTs", name=f"wTs_{sp}")
    nc.vector.tensor_copy(wTs[:, :], wps[0:2, :])
    st.wTs = wTs


def b1_wtp(g, sp):
    """w back to [i,8]; negt; wg = exp(dist*negt); p2 = attn*wg (Pool)."""
    nc, pp = g.nc, g.pp
    st = g.state[sp]
    wtp = pp.tile([P, 8], BF16, tag="ps", name=f"ps_wt{sp}")
    for pi in range(2):
        for it in range(LT):
            c = it * 2 + pi
            nc.tensor.transpose(
                wtp[:, 2 * c:2 * c + 2],
                st.wTs[0:2, pi * L + it * P: pi * L + (it + 1) * P],
                g.id_sb[0:2, 0:2])
    w4 = g.p2p.tile([P, 8], F32, tag="w4", name=f"w4_{sp}")
    nc.scalar.activation(w4[:, :], wtp[:, :], AF.Square, bias=g.b2_sb[:, 0:1])
    nc.vector.tensor_scalar(w4[:, :], w4[:, :], -2.0, -1e-6, ALU.mult, ALU.add)
    negt = g.p2p.tile([P, 8], F32, tag="negt", name=f"negt_{sp}")
    nc.vector.reciprocal(negt[:, :], w4[:, :])
    wg = g.p3.tile([P, FP2], BF16, tag="wg", name=f"wg_{sp}")
    for it in range(LT):
        for pi in range(2):
            c = it * 2 + pi
            sl = slice(it * FP + pi * L, it * FP + (pi + 1) * L)
            nc.scalar.activation(wg[:, sl], g.dist_sb[it][:, :], AF.Exp,
                                 scale=negt[:, 2 * c:2 * c + 1])
    nc.gpsimd.tensor_mul(wg[:, :], st.attn[:, :], wg[:, :])
    st.wg = wg


def b1_sm2(g, sp):
    """E2 = exp(p2); s24 (DVE reduce); attn2 = E2*r24 (Pool)."""
    nc = g.nc
    st = g.state[sp]
    wg = st.wg
    for it in range(LT):
        sl = slice(it * FP, (it + 1) * FP)
        nc.scalar.activation(wg[:, sl], wg[:, sl], AF.Exp)
    s24 = g.p2p.tile([P, 4], F32, tag="s24", name=f"s24_{sp}")
    nc.vector.tensor_reduce(
        s24[:, :], wg[:, :].rearrange("q (c m) -> q c m", c=4),
        axis=mybir.AxisListType.X, op=ALU.add)
    nc.sync.dma_start(out=g.h["s24o"][sp, :, :], in_=s24[:, :])


def b2_t(g, sp):
    """attn2^T -> PSUM bf16; copy to SBUF."""
    nc, pp = g.nc, g.pp
    st = g.state[sp]
    a2ps = pp.tile([P, FP2], BF16, tag="ps", name=f"ps_tB{sp}")
    for mt in range(LT):
        for pi in range(2):
            for it in range(LT):
                nc.tensor.transpose(
                    a2ps[:, mt * FP + pi * L + it * P:
                         mt * FP + pi * L + (it + 1) * P],
                    st.wg[:, it * FP + pi * L + mt * P:
                          it * FP + pi * L + (mt + 1) * P],
                    g.id_sb[:, :])
    a2T = g.p2p.tile([P, FP2], BF16, tag="a2T", name=f"a2T_{sp}")
    nc.vector.tensor_copy(a2T[:, :], a2ps[:, :])
    st.a2T = a2T


def b2_y(g, sp):
    """y = attn2 @ v (8 MMs), copy out bf16, DMA."""
    nc, pp = g.nc, g.pp
    MM = nc.tensor.matmul
    st = g.state[sp]
    for pi in range(2):
        for it in range(LT):
            ps = g.ypp.tile([P, D], F32, tag="psy", name=f"ps_y{sp}_{pi}{it}")
            for mt in range(LT):
                MM(ps[:, :],
                   st.a2T[:, mt * FP + pi * L + it * P:
                          mt * FP + pi * L + (it + 1) * P],
                   st.v[pi][mt][:, :],
                   start=(mt == 0), stop=(mt == LT - 1))
            yt = g.ypool.tile([P, D], BF16, tag=f"y{pi}{it}",
                              name=f"y{sp}_{pi}{it}")
            if (pi + it) % 2 == 0:
                nc.vector.tensor_copy(yt[:, :], ps[:, :])
            else:
                nc.scalar.activation(yt[:, :], ps[:, :], AF.Copy)
            eng = nc.sync if (pi + it) % 2 == 0 else nc.scalar
            eng.dma_start(
                out=g.h["out"][sp, pi * L + it * P: pi * L + (it + 1) * P, :],
                in_=yt[:, :])


def _emit(nc, tc, h):
    import contextlib
    g = _Ctx()
    g.nc, g.h = nc, h
    g.state = {}

    with contextlib.ExitStack() as ex:
        cpool = ex.enter_context(tc.tile_pool(name="consts", bufs=1))
        g.apool = ex.enter_context(tc.tile_pool(name="astream", bufs=3))
        g.vpool = ex.enter_context(tc.tile_pool(name="vstream", bufs=4))
        g.epool = ex.enter_context(tc.tile_pool(name="estream", bufs=4))
        g.p3 = ex.enter_context(tc.tile_pool(name="bstream3", bufs=3))
        g.p2p = ex.enter_context(tc.tile_pool(name="bstream2", bufs=2))
        g.ypool = ex.enter_context(tc.tile_pool(name="yout", bufs=2))
        g.pp = ex.enter_context(tc.tile_pool(name="ps", bufs=6, space="PSUM"))
        g.ypp = ex.enter_context(tc.tile_pool(name="psy", bufs=2, space="PSUM"))

        def cload(name, shape, dt_, src):
            t = cpool.tile(shape, dt_, tag=name, name=name)
            t_all = t[:, :, :] if len(shape) == 3 else t[:, :]
            nc.sync.dma_start(out=t_all, in_=src)
            return t

        def fill(n):
            fp = g.pp.tile([P, FP], F32, tag="ps", name=f"fill{g.nfill}")
            g.nfill += 1
            for _ in range(n):
                nc.tensor.matmul(fp[:, :], g.id_sb[:, :], g.m_sb[0][:, :],
                                 start=True, stop=True)
        g.nfill = 0

        # Identity first, then warmup matmuls during the input-DMA head
        # (lifts the cold HAM pstate while the PE would idle anyway).
        warm_in = cpool.tile([P, P], BF16, tag="warm_in", name="warm_in")
        nc.gpsimd.memset(warm_in[:, :], 1.0)
        g.id_sb = cload("identb", [P, P], BF16, h["identb"][:, :])
        g.id8_sb = cload("ident8", [P, P], F8, h["ident8"][:, :])
        warm_ps = g.pp.tile([P, P], F32, tag="ps", name="warmup_ps")
        for wi in range(28):
            nc.tensor.matmul(warm_ps[:, :], warm_in[:, :], warm_in[:, :],
                             start=True, stop=True)
        g.m_sb = [cload(f"m{dt}", [P, D], BF16, h["m"][dt * P:(dt + 1) * P, :])
                  for dt in range(DT)]

        # ---- pipeline fill ----
        a_xt_half1(g, 0)
        g.wv_sb = [cload(f"wv{dt}", [P, D], BF16,
                         h["wv"][dt * P:(dt + 1) * P, :])
                   for dt in range(DT)]
        g.w1_sb = cload("w1dr", [P, 2, L], F8, h["w1_dr"][:, :, :])
        g.w2_sb = cload("w2dr", [P, 2, P], F8, h["w2_dr"][:, :, :])
        g.b1_sb = cload("b1r", [P, 2], F32, h["b1r"][:, :])
        g.b2_sb = cload("b2r", [P, 1], F32, h["b2r"][:, :])
        g.pos_sb = [cload(f"pos{ci}", [P, FP], BF16, h["posb"][ci, :, :])
                    for ci in range(CH_PER_CORE)]
        g.dist_sb = [cload(f"dist{it}", [P, L], F32, h["distb"][it, :, :])
                     for it in range(LT)]
        a_xt_half2(g, 0)
        a_v(g, 0)
        a_dots(g, 0)
        a_sm1(g, 0)
        a_xt_half1(g, 1)
        a_xt_half2(g, 1)
        a_v(g, 1)
        a_dots(g, 1)
        b1_pool(g, 0)

        # ---- steady-state periods ----
        # Period k PE stream: B1(k) chain steps interleaved with A(k+2)
        # matmul blocks; B2 runs for superpair k-1 (one full period of
        # slack for the softmax2 cross-engine chain).  Engine queues are
        # ordered so ready ops never wait behind blocked ones.
        for k in range(NSP - 1):
            b1_t1(g, k)
            if k + 2 < NSP:
                a_xt_half1(g, k + 2)
            else:
                fill(6)
            if k + 1 < NSP:
                a_sm1(g, k + 1)
            if k >= 1:
                b1_sm2(g, k - 1)
            b1_h(g, k)
            if k + 2 < NSP:
                a_xt_half2(g, k + 2)
            else:
                fill(4)
            if k + 1 < NSP:
                b1_pool(g, k + 1)
            if k + 2 < NSP:
                a_v(g, k + 2)
            else:
                fill(10)
            b1_w(g, k)
            if k >= 1:
                b2_t(g, k - 1)
            b1_wtp(g, k)
            if k + 2 < NSP:
                a_dots(g, k + 2)
            else:
                fill(6)
            if k >= 1:
                b2_y(g, k - 1)

        # ---- last period + drain (hand-scheduled) ----
        q = NSP - 1
        b1_t1(g, q)
        fill(5)
        b1_sm2(g, q - 1)
        b1_h(g, q)
        fill(5)
        b1_w(g, q)
        b2_t(g, q - 1)
        b2_y(g, q - 1)
        b1_wtp(g, q)
        b1_sm2(g, q)
        fill(12)
        b2_t(g, q)
        fill(4)
        b2_y(g, q)


def build_nc():
    nc = bacc.Bacc("TRN2", target_bir_lowering=False, debug=False,
                   enable_asserts=False)
    h = {}
    h["x_t"] = nc.declare_dram_parameter("x_t", [NSP, D, FP], BF16, False)
    h["m"] = nc.declare_dram_parameter("m", [D, D], BF16, False)
    h["wv"] = nc.declare_dram_parameter("wv", [D, D], BF16, False)
    h["w1_dr"] = nc.declare_dram_parameter("w1_dr", [P, 2, L], F8, False)
    h["w2_dr"] = nc.declare_dram_parameter("w2_dr", [P, 2, P], F8, False)
    h["b1r"] = nc.declare_dram_parameter("b1r", [P, 2], F32, False)
    h["b2r"] = nc.declare_dram_parameter("b2r", [P, 1], F32, False)
    h["posb"] = nc.declare_dram_parameter(
        "posb", [CH_PER_CORE, P, FP], BF16, False)
    h["distb"] = nc.declare_dram_parameter("distb", [LT, P, L], F32, False)
    h["identb"] = nc.declare_dram_parameter("identb", [P, P], BF16, False)
    h["ident8"] = nc.declare_dram_parameter("ident8", [P, P], F8, False)
    h["out"] = nc.declare_dram_parameter("out", [NSP, FP, D], BF16, True)
    h["s24o"] = nc.declare_dram_parameter("s24o", [NSP, P, 4], F32, True)

    with TileContext(nc) as tc:
        _emit(nc, tc, h)
    nc.compile()
    return nc


def make_in_maps(x, w_qkv, pos_emb, w1, b1, w2, b2, w_out, b_out):
    import ml_dtypes
    BFNP = ml_dtypes.bfloat16
    F8NP = mybir.dt.np(F8)
    f = lambda a: np.asarray(a, dtype=np.float32)
    x, w_qkv, pos_emb = f(x), f(w_qkv), f(pos_emb)
    w1, b1, w2, b2, w_out, b_out = f(w1), f(b1), f(w2), f(b2), f(w_out), f(b_out)

    wq, wk, wv = w_qkv[:, :D], w_qkv[:, D:2 * D], w_qkv[:, 2 * D:]
    m = (SCALE * (wq.astype(np.float64) @ wk.astype(np.float64).T)
         ).astype(BFNP)
    wvp = (wv.astype(np.float64) @ w_out.astype(np.float64)).astype(BFNP)

    # DoubleRow-packed MLP weights: [p, kk, j] = w[kk*128+p, j]
    w1_dr = np.ascontiguousarray(
        w1.reshape(2, P, L).transpose(1, 0, 2)).astype(F8NP)
    w2_dr = np.zeros((P, 2, P), np.float32)
    w2_dr[:, :, 0:2] = np.broadcast_to(
        w2.reshape(2, P, 1).transpose(1, 0, 2), (P, 2, 2))
    w2_dr = w2_dr.astype(F8NP)
    b1r = np.ascontiguousarray(b1.reshape(2, P).T)
    b2r = np.full((P, 1), b2.reshape(-1)[0], np.float32)

    # pos per channel: [128, (it, m)=512]
    posb_all = pos_emb[0].reshape(C, LT, P, L).transpose(0, 2, 1, 3)
    posb_all = np.ascontiguousarray(posb_all).reshape(C, P, FP).astype(BFNP)

    idx = np.arange(L, dtype=np.float32)
    dist = (idx[None, :] - idx[:, None]) ** 2
    distb = np.ascontiguousarray(dist.reshape(LT, P, L))

    common = {
        "m": np.ascontiguousarray(m),
        "wv": np.ascontiguousarray(wvp),
        "w1_dr": w1_dr,
        "w2_dr": w2_dr,
        "b1r": b1r,
        "b2r": b2r,
        "distb": distb,
        "identb": np.eye(P, dtype=BFNP),
        "ident8": np.eye(P, dtype=F8NP),
    }
    xb = x.astype(BFNP)
    in_maps = []
    for core in range(NCORES):
        x_t = np.empty((NSP, D, FP), BFNP)
        posb = np.empty((CH_PER_CORE, P, FP), BFNP)
        for ci in range(CH_PER_CORE):
            ch = core * CH_PER_CORE + ci
            posb[ci] = posb_all[ch]
            for bp in range(B // 2):
                s = ci * (B // 2) + bp
                x_t[s, :, :L] = xb[2 * bp, ch].T
                x_t[s, :, L:] = xb[2 * bp + 1, ch].T
        mcore = dict(common)
        mcore["x_t"] = x_t
        mcore["posb"] = posb
        in_maps.append(mcore)
    return in_maps


def assemble_out(results, b_out=None):
    """results: per-core dicts with 'out' [NSP, FP, D] (unnormalized
    E2 @ v) and 's24o' [NSP, P, 4] softmax2 row sums; normalize here."""
    y = np.empty((B, C, L, D), np.float32)
    for core in range(NCORES):
        o = np.asarray(results[core]["out"], np.float32)
        s24 = np.asarray(results[core]["s24o"], np.float32)
        # rows i = pi*L + it*P + p  <->  s24[:, p, it*2+pi]
        rs = s24.reshape(NSP, 1, P, 2, 2).transpose(0, 4, 3, 2, 1)
        o = o.reshape(NSP, 2, LT, P, D) / rs
        o = o.reshape(NSP, FP, D)
        for ci in range(CH_PER_CORE):
            ch = core * CH_PER_CORE + ci
            for bp in range(B // 2):
                s = ci * (B // 2) + bp
                y[2 * bp, ch] = o[s, :L, :]
                y[2 * bp + 1, ch] = o[s, L:, :]
    if b_out is not None:
        y += np.asarray(b_out, np.float32).reshape(1, 1, 1, D)
    return y


_NC = None
LAST_RESULT = None


def kernel(x, w_qkv, pos_emb, w1, b1, w2, b2, w_out, b_out):
    global _NC, LAST_RESULT
    from concourse.bass_utils import run_bass_kernel_spmd

    if _NC is None:
        _NC = build_nc()
    in_maps = make_in_maps(x, w_qkv, pos_emb, w1, b1, w2, b2, w_out, b_out)
    res = run_bass_kernel_spmd(_NC, in_maps, core_ids=list(range(NCORES)))
    LAST_RESULT = res
    return assemble_out(res.results, b_out=b_out)


# revision 18
# speedup vs baseline: 1.2524x; 1.0469x over previous
"""Trainium2 Bass kernel: distance-decay double-softmax attention.

Reference computation per (b, c) pair (L=256, D=512):
    qkv  = x @ w_qkv;  q,k,v = split(qkv)
    attn = softmax(q @ k.T * D_h^-0.5)
    h    = relu((attn + pos) @ w1 + b1);  w = h @ w2 + b2
    attn2= softmax(attn * exp(-dist / (2 w^2 + 1e-6)))
    out  = (attn2 @ v) @ w_out + b_out

Host-side algebraic folds (exact):
    dots = q k^T * s = x (s Wq Wk^T) x^T   -> M = s*Wq@Wk.T
    y    = attn2 @ (v w_out) + b_out       -> Wv' = Wv@w_out, b_out on host
    pos streamed raw and added on-device (GpSimd), so no P1 precompute.

Dtype strategy (rel-err budget 2e-2; measured ~3.4e-3 end-to-end):
bf16 for x/M/Wv'/t/E/attn2/v (PE runs bf16 at the same 1 cyc/row as
f32r but with half the LDWEIGHTS cost and half the DMA), fp8e4m3 +
DoubleRow (2x PE rate, K=256 single pass) for the width-MLP whose
effect on the final output is empirically insensitive (4e-5).

Scheduling: the per-superpair B-chain (transpose -> MLP -> negt -> wg
-> softmax2 -> transpose -> y) is a long cross-engine dependency chain.
Every PE step of it is interleaved with independent stage-A matmuls of
superpair sp+2, and the second softmax + B2 run one full period later
than B1, so no PE instruction ever waits on a fresh cross-engine hop.
Emission order per engine is chosen so the in-order DVE/Act queues
never head-of-line block a ready copy behind a waiting reduce.
"""

import sys
import numpy as np

sys.path.insert(0, "/opt/trn_rl_repo")

import concourse.bass as bass  # noqa: E402,F401
import concourse.mybir as mybir  # noqa: E402
from concourse import bacc  # noqa: E402
from concourse.tile import TileContext  # noqa: E402

F32 = mybir.dt.float32
BF16 = mybir.dt.bfloat16
F8 = mybir.dt.float8e4
AF = mybir.ActivationFunctionType
ALU = mybir.AluOpType
DR = mybir.MatmulPerfMode.DoubleRow

B, C, L, D = 8, 16, 256, 512
NCORES = 8
CH_PER_CORE = C // NCORES          # 2
NSP = (B // 2) * CH_PER_CORE       # 8 superpairs per core
P = 128
FP = 2 * L                         # 512: two pairs packed along free dim
FP2 = 2 * FP                       # 1024: both i-tiles packed
DT = D // P                        # 4
LT = L // P                        # 2
SCALE = float(64 ** -0.5)          # DIM_HEAD ** -0.5


class _Ctx:
    pass


# ---------------- stage-A pieces (superpair sp) ----------------

def _tT_group(g, sp, ets):
    nc, pp = g.nc, g.pp
    MM = nc.tensor.matmul
    st = g.state[sp]
    for et in ets:
        ps = pp.tile([P, FP], F32, tag="ps", name=f"ps_t{sp}_{et}")
        for dt in range(DT):
            MM(ps[:, :], g.m_sb[dt][:, et * P:(et + 1) * P], st.xt[dt][:, :],
               start=(dt == 0), stop=(dt == DT - 1))
        t = g.apool.tile([P, FP], BF16, tag=f"tT{et}", name=f"tT{sp}_{et}")
        nc.vector.tensor_copy(t[:, :], ps[:, :])
        st.tT.append(t)


def a_xt_half1(g, sp):
    """xt DMA; first half of t^T = (x M)^T."""
    nc = g.nc
    st = g.state[sp] = _Ctx()
    xt = []
    for dt in range(DT):
        t = g.apool.tile([P, FP], BF16, tag=f"xt{dt}", name=f"xt{sp}_{dt}")
        nc.sync.dma_start(out=t[:, :],
                          in_=g.h["x_t"][sp, dt * P:(dt + 1) * P, :])
        xt.append(t)
    st.xt = xt
    st.tT = []
    _tT_group(g, sp, range(0, DT // 2))


def a_xt_half2(g, sp):
    _tT_group(g, sp, range(DT // 2, DT))


def a_v(g, sp):
    """v = x Wv' (16 MMs) + copies (b_out added on host)."""
    nc, pp = g.nc, g.pp
    MM = nc.tensor.matmul
    st = g.state[sp]
    xt = st.xt
    v_sb = [[None] * LT for _ in range(2)]
    for pi in range(2):
        for lt in range(LT):
            ps = pp.tile([P, D], F32, tag="ps", name=f"ps_v{sp}_{pi}{lt}")
            for dt in range(DT):
                MM(ps[:, :],
                   xt[dt][:, pi * L + lt * P: pi * L + (lt + 1) * P],
                   g.wv_sb[dt][:, :],
                   start=(dt == 0), stop=(dt == DT - 1))
            t = g.vpool.tile([P, D], BF16, tag=f"v{pi}{lt}",
                             name=f"v{sp}_{pi}{lt}")
            if pi == 0:
                nc.scalar.activation(t[:, :], ps[:, :], AF.Copy)
            else:
                nc.vector.tensor_copy(t[:, :], ps[:, :])
            v_sb[pi][lt] = t
    st.v = v_sb


def a_dots(g, sp):
    """dots (16 MMs), E = exp(dots) packed [P,(it,pi,m)], s14, r14."""
    nc, pp = g.nc, g.pp
    MM = nc.tensor.matmul
    st = g.state[sp]
    xt, tT = st.xt, st.tT
    E = g.epool.tile([P, FP2], BF16, tag="E", name=f"E_{sp}")
    for it in range(LT):
        ps = pp.tile([P, FP], F32, tag="ps", name=f"ps_d{sp}_{it}")
        for pi in range(2):
            o = ps[:, pi * L:(pi + 1) * L]
            for et in range(DT):
                MM(o,
                   tT[et][:, pi * L + it * P: pi * L + (it + 1) * P],
                   xt[et][:, pi * L:(pi + 1) * L],
                   start=(et == 0), stop=(et == DT - 1))
        nc.scalar.activation(E[:, it * FP:(it + 1) * FP], ps[:, :], AF.Exp)
    st.E = E


def a_sm1(g, sp):
    """s14 = rowsums(E) (one wide DVE reduce); r14 = 1/s14."""
    nc = g.nc
    st = g.state[sp]
    s14 = g.apool.tile([P, 4], F32, tag="s14", name=f"s14_{sp}")
    nc.vector.tensor_reduce(
        s14[:, :], st.E[:, :].rearrange("q (c m) -> q c m", c=4),
        axis=mybir.AxisListType.X, op=ALU.add)
    r14 = g.apool.tile([P, 4], F32, tag="r14", name=f"r14_{sp}")
    nc.vector.reciprocal(r14[:, :], s14[:, :])
    st.r14 = r14


# ---------------- stage-B pieces ----------------

def _bcast4(r):
    """[P,4] per-(it,pi) scalars -> broadcast AP matching [P,(it,pi,m)]."""
    return r[:, :].unsqueeze(2).broadcast_to((P, 4, L))


def b1_pool(g, sp):
    """attn = E*r1; apw = attn + pos (GpSimd wide ops)."""
    nc = g.nc
    st = g.state[sp]
    ci = sp // (NSP // CH_PER_CORE)
    attn = g.p3.tile([P, FP2], BF16, tag="attn", name=f"attn_{sp}")
    nc.gpsimd.tensor_mul(attn[:, :], st.E[:, :], _bcast4(st.r14))
    st.attn = attn
    apw = g.p3.tile([P, FP2], BF16, tag="apw", name=f"apw_{sp}")
    pos_b = (g.pos_sb[ci][:, :].rearrange("q (a m) -> q a m", a=2)
             .unsqueeze(2).broadcast_to((P, 2, 2, L)))
    nc.gpsimd.tensor_add(apw[:, :], attn[:, :], pos_b)
    st.apw = apw


def b1_t1(g, sp):
    """apw^T -> PSUM bf16 [P,(kk,i)=1024]; copy to fp8 aT8."""
    nc, pp = g.nc, g.pp
    st = g.state[sp]
    aps = pp.tile([P, FP2], BF16, tag="ps", name=f"ps_tA{sp}")
    for mt in range(LT):
        for pi in range(2):
            for it in range(LT):
                nc.tensor.transpose(
                    aps[:, mt * FP + pi * L + it * P:
                        mt * FP + pi * L + (it + 1) * P],
                    st.apw[:, it * FP + pi * L + mt * P:
                           it * FP + pi * L + (mt + 1) * P],
                    g.id_sb[:, :])
    aT8 = g.p2p.tile([P, 2, FP], F8, tag="aT8", name=f"aT8_{sp}")
    nc.vector.tensor_copy(aT8[:, :, :], aps[:, :])
    st.aT8 = aT8


def b1_h(g, sp):
    """h^T = relu(w1^T apw^T + b1): 2 fp8 DoubleRow MMs + Act relu."""
    nc, pp = g.nc, g.pp
    MM = nc.tensor.matmul
    st = g.state[sp]
    hT8 = g.p2p.tile([P, 2, FP], F8, tag="hT8", name=f"hT8_{sp}")
    for jt in range(LT):
        ps = pp.tile([P, FP], F32, tag="ps", name=f"ps_h{sp}_{jt}")
        MM(ps[:, :], g.w1_sb[:, :, jt * P:(jt + 1) * P], st.aT8[:, :, :],
           start=True, stop=True, perf_mode=DR)
        nc.scalar.activation(hT8[:, jt, :], ps[:, :], AF.Relu,
                             bias=g.b1_sb[:, jt:jt + 1])
    st.hT8 = hT8


def b1_w(g, sp):
    """w[i] = h[i,:] @ w2 via DR MMs with hT8 as stationary: out [i, 2]
    per (it, pi) column pair -> wps [128, 8]; then w4/negt directly."""
    nc, pp = g.nc, g.pp
    st = g.state[sp]
    wps = pp.tile([P, 8], F32, tag="ps", name=f"ps_w{sp}")
    for pi in range(2):
        for it in range(LT):
            c = it * 2 + pi
            nc.tensor.matmul(
                wps[:, 2 * c:2 * c + 2],
                st.hT8[:, :, pi * L + it * P: pi * L + (it + 1) * P],
                g.w2_sb[:, :, :],
                start=True, stop=True, perf_mode=DR)
    w4 = g.p2p.tile([P, 8], F32, tag="w4", name=f"w4_{sp}")
    nc.scalar.activation(w4[:, :], wps[:, :], AF.Square, bias=g.b2_sb[:, 0:1])
    nc.vector.tensor_scalar(w4[:, :], w4[:, :], -2.0, -1e-6, ALU.mult, ALU.add)
    negt = g.p2p.tile([P, 8], F32, tag="negt", name=f"negt_{sp}")
    nc.vector.reciprocal(negt[:, :], w4[:, :])
    st.negt = negt


def b1_wtp(g, sp):
    """wg = exp(dist*negt) (Act per-c scale); p2 = attn*wg (Pool)."""
    nc = g.nc
    st = g.state[sp]
    negt = st.negt
    wg = g.p3.tile([P, FP2], BF16, tag="wg", name=f"wg_{sp}")
    for it in range(LT):
        for pi in range(2):
            c = it * 2 + pi
            sl = slice(it * FP + pi * L, it * FP + (pi + 1) * L)
            nc.scalar.activation(wg[:, sl], g.dist_sb[it][:, :], AF.Exp,
                                 scale=negt[:, 2 * c:2 * c + 1])
    nc.gpsimd.tensor_mul(wg[:, :], st.attn[:, :], wg[:, :])
    st.wg = wg


def b1_sm2(g, sp):
    """E2 = exp(p2); s24 (DVE reduce); attn2 = E2*r24 (Pool)."""
    nc = g.nc
    st = g.state[sp]
    wg = st.wg
    for it in range(LT):
        sl = slice(it * FP, (it + 1) * FP)
        nc.scalar.activation(wg[:, sl], wg[:, sl], AF.Exp)
    s24 = g.p2p.tile([P, 4], F32, tag="s24", name=f"s24_{sp}")
    nc.vector.tensor_reduce(
        s24[:, :], wg[:, :].rearrange("q (c m) -> q c m", c=4),
        axis=mybir.AxisListType.X, op=ALU.add)
    nc.sync.dma_start(out=g.h["s24o"][sp, :, :], in_=s24[:, :])


def b2_t(g, sp):
    """attn2^T -> PSUM bf16; copy to SBUF."""
    nc, pp = g.nc, g.pp
    st = g.state[sp]
    a2ps = pp.tile([P, FP2], BF16, tag="ps", name=f"ps_tB{sp}")
    for mt in range(LT):
        for pi in range(2):
            for it in range(LT):
                nc.tensor.transpose(
                    a2ps[:, mt * FP + pi * L + it * P:
                         mt * FP + pi * L + (it + 1) * P],
                    st.wg[:, it * FP + pi * L + mt * P:
                          it * FP + pi * L + (mt + 1) * P],
                    g.id_sb[:, :])
    a2T = g.p2p.tile([P, FP2], BF16, tag="a2T", name=f"a2T_{sp}")
    nc.vector.tensor_copy(a2T[:, :], a2ps[:, :])
    st.a2T = a2T


def b2_y(g, sp):
    """y = attn2 @ v (8 MMs), copy out bf16, DMA."""
    nc, pp = g.nc, g.pp
    MM = nc.tensor.matmul
    st = g.state[sp]
    for pi in range(2):
        for it in range(LT):
            ps = g.ypp.tile([P, D], F32, tag="psy", name=f"ps_y{sp}_{pi}{it}")
            for mt in range(LT):
                MM(ps[:, :],
                   st.a2T[:, mt * FP + pi * L + it * P:
                          mt * FP + pi * L + (it + 1) * P],
                   st.v[pi][mt][:, :],
                   start=(mt == 0), stop=(mt == LT - 1))
            yt = g.ypool.tile([P, D], BF16, tag=f"y{pi}{it}",
                              name=f"y{sp}_{pi}{it}")
            if (pi + it) % 2 == 0:
                nc.vector.tensor_copy(yt[:, :], ps[:, :])
            else:
                nc.scalar.activation(yt[:, :], ps[:, :], AF.Copy)
            eng = nc.sync if (pi + it) % 2 == 0 else nc.scalar
            eng.dma_start(
                out=g.h["out"][sp, pi * L + it * P: pi * L + (it + 1) * P, :],
                in_=yt[:, :])


def _emit(nc, tc, h):
    import contextlib
    g = _Ctx()
    g.nc, g.h = nc, h
    g.state = {}

    with contextlib.ExitStack() as ex:
        cpool = ex.enter_context(tc.tile_pool(name="consts", bufs=1))
        g.apool = ex.enter_context(tc.tile_pool(name="astream", bufs=3))
        g.vpool = ex.enter_context(tc.tile_pool(name="vstream", bufs=4))
        g.epool = ex.enter_context(tc.tile_pool(name="estream", bufs=4))
        g.p3 = ex.enter_context(tc.tile_pool(name="bstream3", bufs=3))
        g.p2p = ex.enter_context(tc.tile_pool(name="bstream2", bufs=2))
        g.ypool = ex.enter_context(tc.tile_pool(name="yout", bufs=2))
        g.pp = ex.enter_context(tc.tile_pool(name="ps", bufs=6, space="PSUM"))
        g.ypp = ex.enter_context(tc.tile_pool(name="psy", bufs=2, space="PSUM"))

        def cload(name, shape, dt_, src):
            t = cpool.tile(shape, dt_, tag=name, name=name)
            t_all = t[:, :, :] if len(shape) == 3 else t[:, :]
            nc.sync.dma_start(out=t_all, in_=src)
            return t

        def fill(n):
            fp = g.pp.tile([P, FP], F32, tag="ps", name=f"fill{g.nfill}")
            g.nfill += 1
            for _ in range(n):
                nc.tensor.matmul(fp[:, :], g.id_sb[:, :], g.m_sb[0][:, :],
                                 start=True, stop=True)
        g.nfill = 0

        # Identity first, then warmup matmuls during the input-DMA head
        # (lifts the cold HAM pstate while the PE would idle anyway).
        warm_in = cpool.tile([P, P], BF16, tag="warm_in", name="warm_in")
        nc.gpsimd.memset(warm_in[:, :], 1.0)
        g.id_sb = cload("identb", [P, P], BF16, h["identb"][:, :])
        g.id8_sb = cload("ident8", [P, P], F8, h["ident8"][:, :])
        warm_ps = g.pp.tile([P, P], F32, tag="ps", name="warmup_ps")
        for wi in range(28):
            nc.tensor.matmul(warm_ps[:, :], warm_in[:, :], warm_in[:, :],
                             start=True, stop=True)
        g.m_sb = [cload(f"m{dt}", [P, D], BF16, h["m"][dt * P:(dt + 1) * P, :])
                  for dt in range(DT)]

        # ---- pipeline fill ----
        a_xt_half1(g, 0)
        g.wv_sb = [cload(f"wv{dt}", [P, D], BF16,
                         h["wv"][dt * P:(dt + 1) * P, :])
                   for dt in range(DT)]
        g.w1_sb = cload("w1dr", [P, 2, L], F8, h["w1_dr"][:, :, :])
        g.w2_sb = cload("w2dr", [P, 2, 2], F8, h["w2_dr"][:, :, :])
        g.b1_sb = cload("b1r", [P, 2], F32, h["b1r"][:, :])
        g.b2_sb = cload("b2r", [P, 1], F32, h["b2r"][:, :])
        g.pos_sb = [cload(f"pos{ci}", [P, FP], BF16, h["posb"][ci, :, :])
                    for ci in range(CH_PER_CORE)]
        g.dist_sb = [cload(f"dist{it}", [P, L], F32, h["distb"][it, :, :])
                     for it in range(LT)]
        a_xt_half2(g, 0)
        a_v(g, 0)
        a_dots(g, 0)
        a_sm1(g, 0)
        a_xt_half1(g, 1)
        a_xt_half2(g, 1)
        a_v(g, 1)
        a_dots(g, 1)
        b1_pool(g, 0)

        # ---- steady-state periods ----
        # Period k PE stream: B1(k) chain steps interleaved with A(k+2)
        # matmul blocks; B2 runs for superpair k-1 (one full period of
        # slack for the softmax2 cross-engine chain).  Engine queues are
        # ordered so ready ops never wait behind blocked ones.
        for k in range(NSP - 1):
            b1_t1(g, k)
            if k + 2 < NSP:
                a_xt_half1(g, k + 2)
            else:
                fill(6)
            if k + 1 < NSP:
                a_sm1(g, k + 1)
            if k >= 1:
                b1_sm2(g, k - 1)
            b1_h(g, k)
            if k + 2 < NSP:
                a_xt_half2(g, k + 2)
            else:
                fill(4)
            if k + 1 < NSP:
                b1_pool(g, k + 1)
            if k + 2 < NSP:
                a_v(g, k + 2)
            else:
                fill(10)
            b1_w(g, k)
            if k >= 1:
                b2_t(g, k - 1)
            b1_wtp(g, k)
            if k + 2 < NSP:
                a_dots(g, k + 2)
            else:
                fill(6)
            if k >= 1:
                b2_y(g, k - 1)

        # ---- last period + drain (hand-scheduled) ----
        q = NSP - 1
        b1_t1(g, q)
        fill(5)
        b1_sm2(g, q - 1)
        b1_h(g, q)
        fill(5)
        b1_w(g, q)
        b2_t(g, q - 1)
        b2_y(g, q - 1)
        b1_wtp(g, q)
        b1_sm2(g, q)
        fill(8)
        b2_t(g, q)
        fill(3)
        b2_y(g, q)


def build_nc():
    nc = bacc.Bacc("TRN2", target_bir_lowering=False, debug=False,
                   enable_asserts=False)
    h = {}
    h["x_t"] = nc.declare_dram_parameter("x_t", [NSP, D, FP], BF16, False)
    h["m"] = nc.declare_dram_parameter("m", [D, D], BF16, False)
    h["wv"] = nc.declare_dram_parameter("wv", [D, D], BF16, False)
    h["w1_dr"] = nc.declare_dram_parameter("w1_dr", [P, 2, L], F8, False)
    h["w2_dr"] = nc.declare_dram_parameter("w2_dr", [P, 2, 2], F8, False)
    h["b1r"] = nc.declare_dram_parameter("b1r", [P, 2], F32, False)
    h["b2r"] = nc.declare_dram_parameter("b2r", [P, 1], F32, False)
    h["posb"] = nc.declare_dram_parameter(
        "posb", [CH_PER_CORE, P, FP], BF16, False)
    h["distb"] = nc.declare_dram_parameter("distb", [LT, P, L], F32, False)
    h["identb"] = nc.declare_dram_parameter("identb", [P, P], BF16, False)
    h["ident8"] = nc.declare_dram_parameter("ident8", [P, P], F8, False)
    h["out"] = nc.declare_dram_parameter("out", [NSP, FP, D], BF16, True)
    h["s24o"] = nc.declare_dram_parameter("s24o", [NSP, P, 4], F32, True)

    with TileContext(nc) as tc:
        _emit(nc, tc, h)
    nc.compile()
    return nc


def make_in_maps(x, w_qkv, pos_emb, w1, b1, w2, b2, w_out, b_out):
    import ml_dtypes
    BFNP = ml_dtypes.bfloat16
    F8NP = mybir.dt.np(F8)
    f = lambda a: np.asarray(a, dtype=np.float32)
    x, w_qkv, pos_emb = f(x), f(w_qkv), f(pos_emb)
    w1, b1, w2, b2, w_out, b_out = f(w1), f(b1), f(w2), f(b2), f(w_out), f(b_out)

    wq, wk, wv = w_qkv[:, :D], w_qkv[:, D:2 * D], w_qkv[:, 2 * D:]
    m = (SCALE * (wq.astype(np.float64) @ wk.astype(np.float64).T)
         ).astype(BFNP)
    wvp = (wv.astype(np.float64) @ w_out.astype(np.float64)).astype(BFNP)

    # DoubleRow-packed MLP weights: [p, kk, j] = w[kk*128+p, j]
    w1_dr = np.ascontiguousarray(
        w1.reshape(2, P, L).transpose(1, 0, 2)).astype(F8NP)
    w2_dr = np.ascontiguousarray(np.broadcast_to(
        w2.reshape(2, P, 1).transpose(1, 0, 2), (P, 2, 2))).astype(F8NP)
    b1r = np.ascontiguousarray(b1.reshape(2, P).T)
    b2r = np.full((P, 1), b2.reshape(-1)[0], np.float32)

    # pos per channel: [128, (it, m)=512]
    posb_all = pos_emb[0].reshape(C, LT, P, L).transpose(0, 2, 1, 3)
    posb_all = np.ascontiguousarray(posb_all).reshape(C, P, FP).astype(BFNP)

    idx = np.arange(L, dtype=np.float32)
    dist = (idx[None, :] - idx[:, None]) ** 2
    distb = np.ascontiguousarray(dist.reshape(LT, P, L))

    common = {
        "m": np.ascontiguousarray(m),
        "wv": np.ascontiguousarray(wvp),
        "w1_dr": w1_dr,
        "w2_dr": w2_dr,
        "b1r": b1r,
        "b2r": b2r,
        "distb": distb,
        "identb": np.eye(P, dtype=BFNP),
        "ident8": np.eye(P, dtype=F8NP),
    }
    xb = x.astype(BFNP)
    in_maps = []
    for core in range(NCORES):
        x_t = np.empty((NSP, D, FP), BFNP)
        posb = np.empty((CH_PER_CORE, P, FP), BFNP)
        for ci in range(CH_PER_CORE):
            ch = core * CH_PER_CORE + ci
            posb[ci] = posb_all[ch]
            for bp in range(B // 2):
                s = ci * (B // 2) + bp
                x_t[s, :, :L] = xb[2 * bp, ch].T
                x_t[s, :, L:] = xb[2 * bp + 1, ch].T
        mcore = dict(common)
        mcore["x_t"] = x_t
        mcore["posb"] = posb
        in_maps.append(mcore)
    return in_maps


def assemble_out(results, b_out=None):
    """results: per-core dicts with 'out' [NSP, FP, D] (unnormalized
    E2 @ v) and 's24o' [NSP, P, 4] softmax2 row sums; normalize here."""
    y = np.empty((B, C, L, D), np.float32)
    for core in range(NCORES):
        o = np.asarray(results[core]["out"], np.float32)
        s24 = np.asarray(results[core]["s24o"], np.float32)
        # rows i = pi*L + it*P + p  <->  s24[:, p, it*2+pi]
        rs = s24.reshape(NSP, 1, P, 2, 2).transpose(0, 4, 3, 2, 1)
        o = o.reshape(NSP, 2, LT, P, D) / rs
        o = o.reshape(NSP, FP, D)
        for ci in range(CH_PER_CORE):
            ch = core * CH_PER_CORE + ci
            for bp in range(B // 2):
                s = ci * (B // 2) + bp
                y[2 * bp, ch] = o[s, :L, :]
                y[2 * bp + 1, ch] = o[s, L:, :]
    if b_out is not None:
        y += np.asarray(b_out, np.float32).reshape(1, 1, 1, D)
    return y


_NC = None
LAST_RESULT = None


def kernel(x, w_qkv, pos_emb, w1, b1, w2, b2, w_out, b_out):
    global _NC, LAST_RESULT
    from concourse.bass_utils import run_bass_kernel_spmd

    if _NC is None:
        _NC = build_nc()
    in_maps = make_in_maps(x, w_qkv, pos_emb, w1, b1, w2, b2, w_out, b_out)
    res = run_bass_kernel_spmd(_NC, in_maps, core_ids=list(range(NCORES)))
    LAST_RESULT = res
    return assemble_out(res.results, b_out=b_out)
